# revision 3
# baseline (speedup 1.0000x reference)
"""ChildSumTreeLSTM (perfect binary tree) Trainium2 kernel.

Problem: B=8 trees, 16384 leaves/tree, D_IN=768, D_H=128; one tree per
NeuronCore (data-parallel over 8 cores), returns (h_root, c_root).

Design (v2; ~225us/iter vs the 312us fp32r/bf16 v1):
  - x and W_in are fp8e4m3; leaf projection runs as DoubleRow matmuls
    (two k-chunks contracted per pass at 0.5 cyc/row) and the DMA floor
    halves.  W_in is stored pre-scaled by 16 to stay out of the fp8
    subnormal range; the 1/16 folds into the PSUM->SBUF gate copy.
  - h state and the gate weights are bf16 (1 cyc/row at any free size,
    no fp32r <256 penalty), c stays fp32.
  - Upper levels pre-sum the child pair on DVE (contiguous moving
    operands, 4 gate matmuls instead of 8).
  - Pieces are 512 wide (half the piece count of v1): PSUM = leaf
    2 banks x 2 bufs + gates (3F + F) = 4 banks single-buffered.
"""

import sys

sys.path.insert(0, "/opt/trn_rl_repo")

import numpy as np

try:
    import jax as _jax

    _jax.config.update("jax_compilation_cache_dir", "/tmp/jax_neff_cache")
    _jax.config.update("jax_persistent_cache_min_compile_time_secs", 10.0)
except Exception:
    pass

import concourse.bass as bass
import concourse.bacc as bacc
import concourse.mybir as mybir
from concourse import tile
from concourse.bass_utils import run_bass_kernel_spmd

AF = mybir.ActivationFunctionType
F32 = mybir.dt.float32
BF16 = mybir.dt.bfloat16
FP8 = mybir.dt.float8e4
DR = mybir.MatmulPerfMode.DoubleRow

N_CORES = 8
D_IN = 768
D_H = 128
N_LEAVES = 16384
KCH = D_IN // 128  # 6 k-chunks
W_IN_SCALE = 16.0  # fp8 W_in pre-scale (undone in the hs copy)


def build_nc(n_leaves=N_LEAVES, f_leaf=1024, f_tree=512, lvl0_f=512,
             xt_bufs=3, reps=1, stop_after=None, taper=False, gate_bufs=2,
             u2_max_f=0, fcs_dve_max_f=256, cs_dve_max_f=64, hs_ps_bufs=2,
             psu_share=True):
    nc = bacc.Bacc("TRN2", target_bir_lowering=False, debug=False)
    n_chunks = n_leaves // f_leaf
    p_chunk = f_leaf // 2  # level-0 parents per leaf chunk

    x_d = nc.dram_tensor("xt", [n_chunks, KCH, 128, f_leaf], FP8, kind="ExternalInput")
    win_d = nc.dram_tensor("w_in", [KCH, 128, D_H], FP8, kind="ExternalInput")
    # w1 block 4 = 2*w1_u, bias rows 8/9 = doubled u biases: tail pieces
    # compute tanh(u) as 2*sigmoid(2u)-1 so all four gates share one
    # sigmoid activation over a contiguous PSUM range.
    w1_d = nc.dram_tensor("w1", [D_H, 5 * D_H], BF16, kind="ExternalInput")
    bias_d = nc.dram_tensor("bias", [10, 128], BF16, kind="ExternalInput")
    ones_d = nc.dram_tensor("ones", [512], BF16, kind="ExternalInput")
    out_d = nc.dram_tensor("out", [2, D_H], F32, kind="ExternalOutput")

    ns = []
    n = n_leaves // 2
    while n >= 1:
        ns.append(n)
        if n == 1:
            break
        n //= 2
    n_levels = len(ns)

    with tile.TileContext(nc) as tc:
        with (
            tc.tile_pool(name="const", bufs=1) as cpool,
            tc.tile_pool(name="state", bufs=1) as bpool,
            tc.tile_pool(name="work", bufs=2) as wpool,
            tc.tile_pool(name="hs_ps", bufs=2, space=bass.MemorySpace.PSUM) as ppool,
            tc.tile_pool(name="g_ps", bufs=1, space=bass.MemorySpace.PSUM) as gpool,
        ):
            w_in = cpool.tile([128, KCH, D_H], FP8, tag="w_in")
            nc.sync.dma_start(out=w_in[:], in_=win_d.rearrange("k p m -> p k m"))
            w1 = cpool.tile([128, 5 * D_H], BF16, tag="w1")
            nc.sync.dma_start(out=w1[:], in_=w1_d[:])
            bias_row_t = cpool.tile([128, 10 * D_H], BF16, tag="bias_row")
            bias_row = bias_row_t[0:1, :]
            nc.sync.dma_start(out=bias_row, in_=bias_d.rearrange("i p -> (i p)"))
            ones_t = cpool.tile([128, 512], BF16, tag="ones")
            ones = ones_t[0:1, :]
            nc.sync.dma_start(out=ones, in_=ones_d.rearrange("(a n) -> a n", a=1))

            # per-level state; h is bf16 (matmul moving operand), c fp32.
            # root h is fp32 (never fed back to a matmul).
            h_buf = [
                bpool.tile([128, ns[l]], BF16 if l < n_levels - 1 else F32,
                           tag=f"h{l}", name=f"h{l}")
                for l in range(n_levels)
            ]
            c_buf = [
                bpool.tile([128, ns[l]], F32, tag=f"c{l}", name=f"c{l}")
                for l in range(n_levels)
            ]

            from concourse.alu_op_type import AluOpType as ALU

            fmax = max(f_tree, lvl0_f)
            # PSUM gate tile: lvl0 merged [i,o](,u2); upper [i,o,f](,u2)
            giof_elems = max(
                (3 if lvl0_f <= u2_max_f else 2) * lvl0_f,
                (4 if f_tree <= u2_max_f else 3) * f_tree,
                4 * min(f_tree, u2_max_f) if u2_max_f else 0,
            )
            gu_elems = max(
                lvl0_f if lvl0_f > u2_max_f else 0,
                f_tree if f_tree > u2_max_f else 0,
            )

            def node_update(F, h_out, c_out, hs, cs=None, lvl0=False):
                """One batch of F parents from the summed-child hidden hs.

                Bulk pieces (F > 256): three-sigmoid merge + separate tanh(u).
                Tail pieces (F <= 256): four gates in ONE sigmoid via
                tanh(u) = 2*sigmoid(2u)-1 (doubled u weights, block 4), and
                the whole elementwise chain on DVE — shorter serial chain.
                """
                nsig = 2 if lvl0 else 3  # merged sigmoid gates: i,o(,f)
                bb = 0 if lvl0 else 4
                tail = F <= u2_max_f

                ps = gpool.tile([128, giof_elems], F32, tag="giof", bufs=gate_bufs)

                def gate_group(dst, g, wg):
                    b = bias_row[:, (bb + g) * D_H : (bb + g + 1) * D_H]
                    nc.tensor.matmul(dst, b, ones[:, 0:F], start=True, stop=False)
                    nc.tensor.matmul(dst, w1[:, wg * D_H : (wg + 1) * D_H], hs,
                                     start=False, stop=True)

                for g in range(nsig):
                    gate_group(ps[:, g * F : (g + 1) * F], g, g)
                a_sig = wpool.tile([128, max(giof_elems, 4 * fmax)], F32, tag="asig")
                u_t = wpool.tile([128, fmax], F32, tag="ut")
                if tail:
                    # u2 block rides the same sigmoid read: bias row 8/9, w1 block 4
                    gate_group(ps[:, nsig * F : (nsig + 1) * F],
                               (8 if lvl0 else 5), 4)  # bb+g -> row 8 / 9
                    nc.scalar.activation(a_sig[:, 0 : (nsig + 1) * F],
                                         ps[:, 0 : (nsig + 1) * F], AF.Sigmoid)
                    nc.vector.tensor_scalar(u_t[:, 0:F],
                                            a_sig[:, nsig * F : (nsig + 1) * F],
                                            2.0, 1.0, ALU.mult, ALU.subtract)
                elif psu_share:
                    # u-gate PSUM rides the leaf pool's bank: frees banks so
                    # the i/o/f block can double-buffer at F=512
                    assert gu_elems <= 512
                    psu = ppool.tile([128, 512], F32, tag="hs_ps",
                                     bufs=hs_ps_bufs, name="psu_sh")
                    gate_group(psu[:, 0:F], 3, 3)
                    nc.scalar.activation(a_sig[:, 0 : nsig * F], ps[:, 0 : nsig * F],
                                         AF.Sigmoid)
                    nc.scalar.activation(u_t[:, 0:F], psu[:, 0:F], AF.Tanh)
                if not tail and not psu_share:
                    psu = gpool.tile([128, gu_elems], F32, tag="gu", bufs=gate_bufs)
                    gate_group(psu[:, 0:F], 3, 3)
                    nc.scalar.activation(a_sig[:, 0 : nsig * F], ps[:, 0 : nsig * F],
                                         AF.Sigmoid)
                    nc.scalar.activation(u_t[:, 0:F], psu[:, 0:F], AF.Tanh)

                i_t = a_sig[:, 0:F]
                o_t = a_sig[:, F : 2 * F]
                if cs is None:  # children carry c == 0
                    nc.vector.tensor_mul(c_out, i_t, u_t[:, 0:F])
                else:
                    f_t = a_sig[:, 2 * F : 3 * F]
                    iu = wpool.tile([128, fmax], F32, tag="iu")
                    nc.vector.tensor_mul(iu[:, 0:F], i_t, u_t[:, 0:F])
                    fcs = wpool.tile([128, fmax], F32, tag="fcs")
                    if F <= fcs_dve_max_f:
                        nc.vector.tensor_mul(fcs[:, 0:F], f_t, cs)
                    else:
                        nc.gpsimd.tensor_mul(fcs[:, 0:F], f_t, cs)
                    nc.vector.tensor_add(c_out, iu[:, 0:F], fcs[:, 0:F])
                t = wpool.tile([128, fmax], F32, tag="t")
                nc.scalar.activation(t[:, 0:F], c_out, AF.Tanh)
                nc.vector.tensor_mul(h_out, o_t, t[:, 0:F])

            def emit_tree_chunk(l, j0, F):
                """Level-l parents [j0, j0+F) from level l-1 children."""
                hv = h_buf[l - 1].rearrange("p (n two) -> p n two", two=2)
                cv = c_buf[l - 1].rearrange("p (n two) -> p n two", two=2)
                cs = wpool.tile([128, fmax], F32, tag="cs")
                cs_eng = nc.vector if F <= cs_dve_max_f else nc.gpsimd
                cs_eng.tensor_add(cs[:, 0:F], cv[:, j0 : j0 + F, 0],
                                  cv[:, j0 : j0 + F, 1])
                hsum = wpool.tile([128, fmax], BF16, tag="hsum")
                nc.vector.tensor_add(hsum[:, 0:F], hv[:, j0 : j0 + F, 0],
                                     hv[:, j0 : j0 + F, 1])
                node_update(F, h_buf[l][:, j0 : j0 + F], c_buf[l][:, j0 : j0 + F],
                            hs=hsum[:, 0:F], cs=cs[:, 0:F])

            emitted = [0] * n_levels

            def level_pieces(l, n):
                """Bulk f_tree pieces plus a final ancestor-cone piece: the
                last-emitted pieces form the kernel's serial tail, so the
                final piece per level shrinks geometrically."""
                cone = max((f_leaf // 2) >> l, 1)
                if (not taper) or n <= max(cone, 32):
                    cone = 0
                out = []
                rem = n - cone
                while rem > f_tree:
                    out.append(f_tree)
                    rem -= f_tree
                if rem:
                    out.append(rem)
                if cone:
                    out.append(cone)
                return out

            piece_plan = [None] + [level_pieces(l, ns[l]) for l in range(1, n_levels)]
            piece_idx = [0] * n_levels

            def collect_ready():
                """Pop pieces whose inputs were complete in prior batches."""
                snap = list(emitted)
                out = []
                for l in range(1, n_levels):
                    plan = piece_plan[l]
                    while piece_idx[l] < len(plan):
                        Fl = plan[piece_idx[l]]
                        if 2 * (emitted[l] + Fl) > snap[l - 1]:
                            break
                        out.append((l, emitted[l], Fl))
                        emitted[l] += Fl
                        piece_idx[l] += 1
                return out

            def _emit_main():
              hs_ring = {}
              ready = []
              emitted[:] = [0] * n_levels
              piece_idx[:] = [0] * n_levels
              for ci in range(n_chunks + 1):
                if ci < n_chunks:
                    xt = wpool.tile([128, KCH, f_leaf], FP8, tag="xt", bufs=xt_bufs)
                    xv = xt.rearrange("p k (n two) -> p k n two", two=2)
                    # PSUM groups are bank-limited to 512 fp32: split the
                    # chunk's parents into 512-wide groups, each fed by all
                    # k-pair DoubleRow matmuls.
                    n_ph = p_chunk // 512
                    hs_psl = [ppool.tile([128, 512], F32, tag="hs_ps",
                                         bufs=hs_ps_bufs, name=f"hs_ps{ph}")
                              for ph in range(n_ph)]
                    hs = wpool.tile([128, p_chunk], BF16, tag="hs", bufs=3)
                    for kp in range(KCH // 2):
                        nc.sync.dma_start(
                            out=xt[:, 2 * kp : 2 * kp + 2, :],
                            in_=x_d[ci][2 * kp : 2 * kp + 2].rearrange("k p n -> p k n"),
                        )
                        for ph in range(n_ph):
                            sl = slice(ph * 512, (ph + 1) * 512)
                            for two in range(2):
                                nc.tensor.matmul(
                                    hs_psl[ph][:],
                                    w_in[:, 2 * kp : 2 * kp + 2, :],
                                    xv[:, 2 * kp : 2 * kp + 2, sl, two],
                                    start=(kp == 0 and two == 0),
                                    stop=(kp == KCH // 2 - 1 and two == 1),
                                    perf_mode=DR,
                                )
                    for ph in range(n_ph):
                        sl = slice(ph * 512, (ph + 1) * 512)
                        # undo the fp8 W_in pre-scale while leaving PSUM
                        nc.vector.tensor_scalar_mul(hs[:, sl], hs_psl[ph][:],
                                                    1.0 / W_IN_SCALE)
                    hs_ring[ci] = hs
                if ci >= 1 and stop_after != "leaf":  # lagged level-0 update
                    cj = ci - 1
                    hs_t = hs_ring.pop(cj)
                    f0 = min(lvl0_f, p_chunk)
                    for s in range(p_chunk // f0):
                        j0 = cj * p_chunk + s * f0
                        node_update(
                            f0, h_buf[0][:, j0 : j0 + f0], c_buf[0][:, j0 : j0 + f0],
                            hs=hs_t[:, s * f0 : (s + 1) * f0], lvl0=True,
                        )
                    emitted[0] += p_chunk
                if stop_after is None:
                    for l, j0, Fl in ready:  # lagged cascade pieces
                        emit_tree_chunk(l, j0, Fl)
                    ready = collect_ready()
              if stop_after == "leaf":
                  last = hs_ring[n_chunks - 1]
                  nc.sync.dma_start(
                      out=out_d[0:1, :].rearrange("a p -> p a"),
                      in_=last[:, 0:1].bitcast(F32),
                  )
                  return
              if stop_after == "lvl0":
                  nc.sync.dma_start(
                      out=out_d[0:1, :].rearrange("a p -> p a"),
                      in_=h_buf[0][:, 0:1].bitcast(F32),
                  )
                  nc.sync.dma_start(
                      out=out_d[1:2, :].rearrange("a p -> p a"), in_=c_buf[0][:, 0:1]
                  )
                  return
              while ready:
                for l, j0, Fl in ready:
                    emit_tree_chunk(l, j0, Fl)
                ready = collect_ready()

              assert all(emitted[l] == ns[l] for l in range(n_levels)), emitted

              nc.sync.dma_start(out=out_d[0:1, :].rearrange("a p -> p a"), in_=h_buf[-1][:])
              nc.sync.dma_start(out=out_d[1:2, :].rearrange("a p -> p a"), in_=c_buf[-1][:])

            if reps == 1:
                _emit_main()
            elif reps < 0:  # unrolled (sim-friendly) repetition
                for _ in range(-reps):
                    _emit_main()
            else:
                with tc.For_i(0, reps, 1):
                    _emit_main()

    nc.compile()
    return nc


# W_up/bias gate permutation [i, o, u, f] -> [i, o, f, u]
_GPERM = (0, 1, 3, 2)


def prep_inputs(x, W_in, b_in, W_up, b_up, n_leaves=N_LEAVES, f_leaf=1024):
    import ml_dtypes

    fp8 = ml_dtypes.float8_e4m3
    bf16 = ml_dtypes.bfloat16
    x = np.asarray(x, dtype=np.float32)
    W_in = np.asarray(W_in, dtype=np.float32)
    b_in = np.asarray(b_in, dtype=np.float32)
    W_up = np.asarray(W_up, dtype=np.float32)
    b_up = np.asarray(b_up, dtype=np.float32)

    n_chunks = n_leaves // f_leaf
    w1g = (0.5 * W_up).reshape(D_H, 4, D_H)[:, _GPERM, :]
    w1 = w1g.reshape(D_H, 4 * D_H)
    # block 4 = doubled u weights (tail pieces compute tanh(u)=2*sig(2u)-1)
    w1 = np.ascontiguousarray(
        np.concatenate([w1, 2.0 * w1g[:, 3, :]], axis=1)
    ).astype(bf16)
    bias0 = (b_in @ W_up + b_up).reshape(4, D_H)[_GPERM, :]
    biasr = b_up.reshape(4, D_H)[_GPERM, :]
    bias_h = np.ascontiguousarray(
        np.concatenate([bias0, biasr, 2.0 * bias0[3:4], 2.0 * biasr[3:4]])
    ).astype(bf16)
    win_h = np.ascontiguousarray(
        (W_in * W_IN_SCALE).reshape(KCH, 128, D_H)
    ).astype(fp8)

    in_maps = []
    for i in range(x.shape[0]):
        xt = np.ascontiguousarray(
            x[i].T.reshape(KCH, 128, n_chunks, f_leaf).transpose(2, 0, 1, 3)
        ).astype(fp8)
        in_maps.append({"xt": xt, "w_in": win_h, "w1": w1, "bias": bias_h,
                        "ones": np.ones(512, bf16)})
    return in_maps


_NC_CACHE = {}


def kernel(x, W_in, b_in, W_up, b_up):
    x = np.asarray(x, dtype=np.float32)
    B = x.shape[0]
    assert B == N_CORES and x.shape[1] == N_LEAVES and x.shape[2] == D_IN

    if N_LEAVES not in _NC_CACHE:
        _NC_CACHE[N_LEAVES] = build_nc(N_LEAVES)
    nc = _NC_CACHE[N_LEAVES]

    in_maps = prep_inputs(x, W_in, b_in, W_up, b_up)
    res = run_bass_kernel_spmd(nc, in_maps, list(range(N_CORES)))
    out = np.stack([res.results[i]["out"] for i in range(N_CORES)])
    return out[:, 0].astype(np.float32), out[:, 1].astype(np.float32)


# revision 4
# speedup vs baseline: 1.1197x; 1.1197x over previous
"""ChildSumTreeLSTM (perfect binary tree) Trainium2 kernel.

Problem: B=8 trees, 16384 leaves/tree, D_IN=768, D_H=128; one tree per
NeuronCore (data-parallel over 8 cores), returns (h_root, c_root).

Design (v2; ~225us/iter vs the 312us fp32r/bf16 v1):
  - x and W_in are fp8e4m3; leaf projection runs as DoubleRow matmuls
    (two k-chunks contracted per pass at 0.5 cyc/row) and the DMA floor
    halves.  W_in is stored pre-scaled by 16 to stay out of the fp8
    subnormal range; the 1/16 folds into the PSUM->SBUF gate copy.
  - h state and the gate weights are bf16 (1 cyc/row at any free size,
    no fp32r <256 penalty), c stays fp32.
  - Upper levels pre-sum the child pair on DVE (contiguous moving
    operands, 4 gate matmuls instead of 8).
  - Pieces are 512 wide (half the piece count of v1): PSUM = leaf
    2 banks x 2 bufs + gates (3F + F) = 4 banks single-buffered.
"""

import sys

sys.path.insert(0, "/opt/trn_rl_repo")

import numpy as np

try:
    import jax as _jax

    _jax.config.update("jax_compilation_cache_dir", "/tmp/jax_neff_cache")
    _jax.config.update("jax_persistent_cache_min_compile_time_secs", 10.0)
except Exception:
    pass

import concourse.bass as bass
import concourse.bacc as bacc
import concourse.mybir as mybir
from concourse import tile
from concourse.bass_utils import run_bass_kernel_spmd

AF = mybir.ActivationFunctionType
F32 = mybir.dt.float32
BF16 = mybir.dt.bfloat16
FP8 = mybir.dt.float8e4
DR = mybir.MatmulPerfMode.DoubleRow

N_CORES = 8
D_IN = 768
D_H = 128
N_LEAVES = 16384
KCH = D_IN // 128  # 6 k-chunks
W_IN_SCALE = 16.0  # fp8 W_in pre-scale (undone in the hs copy)


def build_nc(n_leaves=N_LEAVES, f_leaf=2048, f_tree=512, lvl0_f=512,
             xt_bufs=3, reps=1, stop_after=None, taper=False, gate_bufs=2,
             u2_max_f=0, fcs_dve_max_f=256, cs_dve_max_f=64, hs_ps_bufs=2,
             psu_share=True):
    nc = bacc.Bacc("TRN2", target_bir_lowering=False, debug=False)
    n_chunks = n_leaves // f_leaf
    p_chunk = f_leaf // 2  # level-0 parents per leaf chunk

    x_d = nc.dram_tensor("xt", [n_chunks, KCH, 128, f_leaf], FP8, kind="ExternalInput")
    win_d = nc.dram_tensor("w_in", [KCH, 128, D_H], FP8, kind="ExternalInput")
    # w1 block 4 = 2*w1_u, bias rows 8/9 = doubled u biases: tail pieces
    # compute tanh(u) as 2*sigmoid(2u)-1 so all four gates share one
    # sigmoid activation over a contiguous PSUM range.
    w1_d = nc.dram_tensor("w1", [D_H, 5 * D_H], BF16, kind="ExternalInput")
    bias_d = nc.dram_tensor("bias", [10, 128], BF16, kind="ExternalInput")
    ones_d = nc.dram_tensor("ones", [512], BF16, kind="ExternalInput")
    out_d = nc.dram_tensor("out", [2, D_H], F32, kind="ExternalOutput")

    ns = []
    n = n_leaves // 2
    while n >= 1:
        ns.append(n)
        if n == 1:
            break
        n //= 2
    n_levels = len(ns)

    with tile.TileContext(nc) as tc:
        with (
            tc.tile_pool(name="const", bufs=1) as cpool,
            tc.tile_pool(name="state", bufs=1) as bpool,
            tc.tile_pool(name="work", bufs=2) as wpool,
            tc.tile_pool(name="hs_ps", bufs=2, space=bass.MemorySpace.PSUM) as ppool,
            tc.tile_pool(name="g_ps", bufs=1, space=bass.MemorySpace.PSUM) as gpool,
        ):
            w_in = cpool.tile([128, KCH, D_H], FP8, tag="w_in")
            nc.sync.dma_start(out=w_in[:], in_=win_d.rearrange("k p m -> p k m"))
            w1 = cpool.tile([128, 5 * D_H], BF16, tag="w1")
            nc.sync.dma_start(out=w1[:], in_=w1_d[:])
            bias_row_t = cpool.tile([128, 10 * D_H], BF16, tag="bias_row")
            bias_row = bias_row_t[0:1, :]
            nc.sync.dma_start(out=bias_row, in_=bias_d.rearrange("i p -> (i p)"))
            ones_t = cpool.tile([128, 512], BF16, tag="ones")
            ones = ones_t[0:1, :]
            nc.sync.dma_start(out=ones, in_=ones_d.rearrange("(a n) -> a n", a=1))

            # per-level state; h is bf16 (matmul moving operand), c fp32.
            # root h is fp32 (never fed back to a matmul).
            h_buf = [
                bpool.tile([128, ns[l]], BF16 if l < n_levels - 1 else F32,
                           tag=f"h{l}", name=f"h{l}")
                for l in range(n_levels)
            ]
            c_buf = [
                bpool.tile([128, ns[l]], F32, tag=f"c{l}", name=f"c{l}")
                for l in range(n_levels)
            ]

            from concourse.alu_op_type import AluOpType as ALU

            fmax = max(f_tree, lvl0_f)
            # PSUM gate tile: lvl0 merged [i,o](,u2); upper [i,o,f](,u2)
            giof_elems = max(
                (3 if lvl0_f <= u2_max_f else 2) * lvl0_f,
                (4 if f_tree <= u2_max_f else 3) * f_tree,
                4 * min(f_tree, u2_max_f) if u2_max_f else 0,
            )
            gu_elems = max(
                lvl0_f if lvl0_f > u2_max_f else 0,
                f_tree if f_tree > u2_max_f else 0,
            )

            def node_update(F, h_out, c_out, hs, cs=None, lvl0=False):
                """One batch of F parents from the summed-child hidden hs.

                Bulk pieces (F > 256): three-sigmoid merge + separate tanh(u).
                Tail pieces (F <= 256): four gates in ONE sigmoid via
                tanh(u) = 2*sigmoid(2u)-1 (doubled u weights, block 4), and
                the whole elementwise chain on DVE — shorter serial chain.
                """
                nsig = 2 if lvl0 else 3  # merged sigmoid gates: i,o(,f)
                bb = 0 if lvl0 else 4
                tail = F <= u2_max_f

                ps = gpool.tile([128, giof_elems], F32, tag="giof", bufs=gate_bufs)

                def gate_group(dst, g, wg):
                    b = bias_row[:, (bb + g) * D_H : (bb + g + 1) * D_H]
                    nc.tensor.matmul(dst, b, ones[:, 0:F], start=True, stop=False)
                    nc.tensor.matmul(dst, w1[:, wg * D_H : (wg + 1) * D_H], hs,
                                     start=False, stop=True)

                for g in range(nsig):
                    gate_group(ps[:, g * F : (g + 1) * F], g, g)
                a_sig = wpool.tile([128, max(giof_elems, 4 * fmax)], F32, tag="asig")
                u_t = wpool.tile([128, fmax], F32, tag="ut")
                if tail:
                    # u2 block rides the same sigmoid read: bias row 8/9, w1 block 4
                    gate_group(ps[:, nsig * F : (nsig + 1) * F],
                               (8 if lvl0 else 5), 4)  # bb+g -> row 8 / 9
                    nc.scalar.activation(a_sig[:, 0 : (nsig + 1) * F],
                                         ps[:, 0 : (nsig + 1) * F], AF.Sigmoid)
                    nc.vector.tensor_scalar(u_t[:, 0:F],
                                            a_sig[:, nsig * F : (nsig + 1) * F],
                                            2.0, 1.0, ALU.mult, ALU.subtract)
                elif psu_share:
                    # u-gate PSUM rides the leaf pool's bank: frees banks so
                    # the i/o/f block can double-buffer at F=512
                    assert gu_elems <= 512
                    psu = ppool.tile([128, 512], F32, tag="hs_ps",
                                     bufs=hs_ps_bufs, name="psu_sh")
                    gate_group(psu[:, 0:F], 3, 3)
                    nc.scalar.activation(a_sig[:, 0 : nsig * F], ps[:, 0 : nsig * F],
                                         AF.Sigmoid)
                    nc.scalar.activation(u_t[:, 0:F], psu[:, 0:F], AF.Tanh)
                if not tail and not psu_share:
                    psu = gpool.tile([128, gu_elems], F32, tag="gu", bufs=gate_bufs)
                    gate_group(psu[:, 0:F], 3, 3)
                    nc.scalar.activation(a_sig[:, 0 : nsig * F], ps[:, 0 : nsig * F],
                                         AF.Sigmoid)
                    nc.scalar.activation(u_t[:, 0:F], psu[:, 0:F], AF.Tanh)

                i_t = a_sig[:, 0:F]
                o_t = a_sig[:, F : 2 * F]
                if cs is None:  # children carry c == 0
                    nc.vector.tensor_mul(c_out, i_t, u_t[:, 0:F])
                else:
                    f_t = a_sig[:, 2 * F : 3 * F]
                    iu = wpool.tile([128, fmax], F32, tag="iu")
                    nc.vector.tensor_mul(iu[:, 0:F], i_t, u_t[:, 0:F])
                    fcs = wpool.tile([128, fmax], F32, tag="fcs")
                    if F <= fcs_dve_max_f:
                        nc.vector.tensor_mul(fcs[:, 0:F], f_t, cs)
                    else:
                        nc.gpsimd.tensor_mul(fcs[:, 0:F], f_t, cs)
                    nc.vector.tensor_add(c_out, iu[:, 0:F], fcs[:, 0:F])
                t = wpool.tile([128, fmax], F32, tag="t")
                nc.scalar.activation(t[:, 0:F], c_out, AF.Tanh)
                nc.vector.tensor_mul(h_out, o_t, t[:, 0:F])

            def emit_tree_chunk(l, j0, F):
                """Level-l parents [j0, j0+F) from level l-1 children."""
                hv = h_buf[l - 1].rearrange("p (n two) -> p n two", two=2)
                cv = c_buf[l - 1].rearrange("p (n two) -> p n two", two=2)
                cs = wpool.tile([128, fmax], F32, tag="cs")
                cs_eng = nc.vector if F <= cs_dve_max_f else nc.gpsimd
                cs_eng.tensor_add(cs[:, 0:F], cv[:, j0 : j0 + F, 0],
                                  cv[:, j0 : j0 + F, 1])
                hsum = wpool.tile([128, fmax], BF16, tag="hsum")
                nc.vector.tensor_add(hsum[:, 0:F], hv[:, j0 : j0 + F, 0],
                                     hv[:, j0 : j0 + F, 1])
                node_update(F, h_buf[l][:, j0 : j0 + F], c_buf[l][:, j0 : j0 + F],
                            hs=hsum[:, 0:F], cs=cs[:, 0:F])

            emitted = [0] * n_levels

            def level_pieces(l, n):
                """Bulk f_tree pieces plus a final ancestor-cone piece: the
                last-emitted pieces form the kernel's serial tail, so the
                final piece per level shrinks geometrically."""
                cone = max((f_leaf // 2) >> l, 1)
                if (not taper) or n <= max(cone, 32):
                    cone = 0
                out = []
                rem = n - cone
                while rem > f_tree:
                    out.append(f_tree)
                    rem -= f_tree
                if rem:
                    out.append(rem)
                if cone:
                    out.append(cone)
                return out

            piece_plan = [None] + [level_pieces(l, ns[l]) for l in range(1, n_levels)]
            piece_idx = [0] * n_levels

            def collect_ready():
                """Pop pieces whose inputs were complete in prior batches."""
                snap = list(emitted)
                out = []
                for l in range(1, n_levels):
                    plan = piece_plan[l]
                    while piece_idx[l] < len(plan):
                        Fl = plan[piece_idx[l]]
                        if 2 * (emitted[l] + Fl) > snap[l - 1]:
                            break
                        out.append((l, emitted[l], Fl))
                        emitted[l] += Fl
                        piece_idx[l] += 1
                return out

            def _emit_main():
              hs_ring = {}
              ready = []
              emitted[:] = [0] * n_levels
              piece_idx[:] = [0] * n_levels
              for ci in range(n_chunks + 1):
                if ci < n_chunks:
                    xt = wpool.tile([128, KCH, f_leaf], FP8, tag="xt", bufs=xt_bufs)
                    xv = xt.rearrange("p k (n two) -> p k n two", two=2)
                    # PSUM groups are bank-limited to 512 fp32: split the
                    # chunk's parents into 512-wide groups, each fed by all
                    # k-pair DoubleRow matmuls.
                    n_ph = p_chunk // 512
                    hs_psl = [ppool.tile([128, 512], F32, tag="hs_ps",
                                         bufs=hs_ps_bufs, name=f"hs_ps{ph}")
                              for ph in range(n_ph)]
                    hs = wpool.tile([128, p_chunk], BF16, tag="hs", bufs=3)
                    for kp in range(KCH // 2):
                        nc.sync.dma_start(
                            out=xt[:, 2 * kp : 2 * kp + 2, :],
                            in_=x_d[ci][2 * kp : 2 * kp + 2].rearrange("k p n -> p k n"),
                        )
                        for ph in range(n_ph):
                            sl = slice(ph * 512, (ph + 1) * 512)
                            for two in range(2):
                                nc.tensor.matmul(
                                    hs_psl[ph][:],
                                    w_in[:, 2 * kp : 2 * kp + 2, :],
                                    xv[:, 2 * kp : 2 * kp + 2, sl, two],
                                    start=(kp == 0 and two == 0),
                                    stop=(kp == KCH // 2 - 1 and two == 1),
                                    perf_mode=DR,
                                )
                    for ph in range(n_ph):
                        sl = slice(ph * 512, (ph + 1) * 512)
                        # undo the fp8 W_in pre-scale while leaving PSUM
                        nc.vector.tensor_scalar_mul(hs[:, sl], hs_psl[ph][:],
                                                    1.0 / W_IN_SCALE)
                    hs_ring[ci] = hs
                if ci >= 1 and stop_after != "leaf":  # lagged level-0 update
                    cj = ci - 1
                    hs_t = hs_ring.pop(cj)
                    f0 = min(lvl0_f, p_chunk)
                    for s in range(p_chunk // f0):
                        j0 = cj * p_chunk + s * f0
                        node_update(
                            f0, h_buf[0][:, j0 : j0 + f0], c_buf[0][:, j0 : j0 + f0],
                            hs=hs_t[:, s * f0 : (s + 1) * f0], lvl0=True,
                        )
                    emitted[0] += p_chunk
                if stop_after is None:
                    for l, j0, Fl in ready:  # lagged cascade pieces
                        emit_tree_chunk(l, j0, Fl)
                    ready = collect_ready()
              if stop_after == "leaf":
                  last = hs_ring[n_chunks - 1]
                  nc.sync.dma_start(
                      out=out_d[0:1, :].rearrange("a p -> p a"),
                      in_=last[:, 0:1].bitcast(F32),
                  )
                  return
              if stop_after == "lvl0":
                  nc.sync.dma_start(
                      out=out_d[0:1, :].rearrange("a p -> p a"),
                      in_=h_buf[0][:, 0:1].bitcast(F32),
                  )
                  nc.sync.dma_start(
                      out=out_d[1:2, :].rearrange("a p -> p a"), in_=c_buf[0][:, 0:1]
                  )
                  return
              while ready:
                for l, j0, Fl in ready:
                    emit_tree_chunk(l, j0, Fl)
                ready = collect_ready()

              assert all(emitted[l] == ns[l] for l in range(n_levels)), emitted

              nc.sync.dma_start(out=out_d[0:1, :].rearrange("a p -> p a"), in_=h_buf[-1][:])
              nc.sync.dma_start(out=out_d[1:2, :].rearrange("a p -> p a"), in_=c_buf[-1][:])

            if reps == 1:
                _emit_main()
            elif reps < 0:  # unrolled (sim-friendly) repetition
                for _ in range(-reps):
                    _emit_main()
            else:
                with tc.For_i(0, reps, 1):
                    _emit_main()

    nc.compile()
    return nc


# W_up/bias gate permutation [i, o, u, f] -> [i, o, f, u]
_GPERM = (0, 1, 3, 2)


def prep_inputs(x, W_in, b_in, W_up, b_up, n_leaves=N_LEAVES, f_leaf=2048):
    import ml_dtypes

    fp8 = ml_dtypes.float8_e4m3
    bf16 = ml_dtypes.bfloat16
    x = np.asarray(x, dtype=np.float32)
    W_in = np.asarray(W_in, dtype=np.float32)
    b_in = np.asarray(b_in, dtype=np.float32)
    W_up = np.asarray(W_up, dtype=np.float32)
    b_up = np.asarray(b_up, dtype=np.float32)

    n_chunks = n_leaves // f_leaf
    w1g = (0.5 * W_up).reshape(D_H, 4, D_H)[:, _GPERM, :]
    w1 = w1g.reshape(D_H, 4 * D_H)
    # block 4 = doubled u weights (tail pieces compute tanh(u)=2*sig(2u)-1)
    w1 = np.ascontiguousarray(
        np.concatenate([w1, 2.0 * w1g[:, 3, :]], axis=1)
    ).astype(bf16)
    bias0 = (b_in @ W_up + b_up).reshape(4, D_H)[_GPERM, :]
    biasr = b_up.reshape(4, D_H)[_GPERM, :]
    bias_h = np.ascontiguousarray(
        np.concatenate([bias0, biasr, 2.0 * bias0[3:4], 2.0 * biasr[3:4]])
    ).astype(bf16)
    win_h = np.ascontiguousarray(
        (W_in * W_IN_SCALE).reshape(KCH, 128, D_H)
    ).astype(fp8)

    in_maps = []
    for i in range(x.shape[0]):
        xt = np.ascontiguousarray(
            x[i].T.reshape(KCH, 128, n_chunks, f_leaf).transpose(2, 0, 1, 3)
        ).astype(fp8)
        in_maps.append({"xt": xt, "w_in": win_h, "w1": w1, "bias": bias_h,
                        "ones": np.ones(512, bf16)})
    return in_maps


_NC_CACHE = {}


def kernel(x, W_in, b_in, W_up, b_up):
    x = np.asarray(x, dtype=np.float32)
    B = x.shape[0]
    assert B == N_CORES and x.shape[1] == N_LEAVES and x.shape[2] == D_IN

    if N_LEAVES not in _NC_CACHE:
        _NC_CACHE[N_LEAVES] = build_nc(N_LEAVES)
    nc = _NC_CACHE[N_LEAVES]

    in_maps = prep_inputs(x, W_in, b_in, W_up, b_up)
    res = run_bass_kernel_spmd(nc, in_maps, list(range(N_CORES)))
    out = np.stack([res.results[i]["out"] for i in range(N_CORES)])
    return out[:, 0].astype(np.float32), out[:, 1].astype(np.float32)
